# revision 14
# baseline (speedup 1.0000x reference)
"""Trainium2 Bass kernel for nn_Attention (B=8, N=1024, D=768, H=12).

Strategy: pure data-parallel over batch - core b computes the full attention
layer for batch element b. No collectives.

v2 schedule (vs v1 baseline):
  - x loaded once in bf16 (host-cast); LN stats via ones-matmuls on bf16,
    rstd fused into one ScalarE Rsqrt activation.
  - scores row-tiled: head pair (2h, 2h+1) runs as two concurrent K=64
    matmuls in PE row-groups (0,0)/(64,0) -> ~2x scores throughput.
  - one [128, 2048] PSUM scores tile per (pair, kt) holds both heads'
    scoresT; a single ScalarE Exp covers the pair (48 instead of 96 exps).
  - emission interleaves qk/v projection "filler" units between score
    groups so the PE stream never idles long enough to re-throttle (HAM).
  - AV deferred by one pair (at ring bufs=2) so fillers have two score
    windows to drain; denominators ride the AV matmul (M=65), staged to
    32-aligned partitions, one batched DVE reciprocal per pair, K=1
    broadcast matmuls with explicit tile_position (no DMA staging).
  - out projection double-buffered after heads' PSUM pools close.
"""

import json
import os
import sys

sys.path.insert(0, "/opt/trn_rl_repo")

import numpy as np
import ml_dtypes

bf16 = ml_dtypes.bfloat16

B, N, D = 8, 1024, 768
H, DH = 12, 64
KT = D // 128          # 6 k-tiles over the model dim
NT = N // 128          # 8 tiles over seq
NCH = N // 512         # 2 free-dim chunks of 512
NP = H // 2            # 6 head pairs
F32 = np.float32

_cache = {}


# ---------------------------------------------------------------------------
# Workaround: this walrus build rejects >1 sync wait per instruction. Split
# excess waits onto same-engine NoOps inserted just before the instruction
# (in-order per engine, so semantics are unchanged).
# ---------------------------------------------------------------------------
def _install_ntff_hook():
    """Provide antenv.axon_hooks if the image lacks it, so trace=True /
    BASS_TRACE=1 can capture NTFF profiles via libaxon_pjrt.so."""
    import types
    import contextlib
    import ctypes

    try:
        import antenv.axon_hooks  # noqa: F401
        return
    except ImportError:
        pass
    import antenv

    mod = types.ModuleType("antenv.axon_hooks")
    holder = [None]
    mod.set_axon_ntff_profile_hook = lambda h: holder.__setitem__(0, h)
    mod.get_axon_ntff_profile_hook = lambda: holder[0]
    sys.modules["antenv.axon_hooks"] = mod
    antenv.axon_hooks = mod

    so_path = "/opt/axon/libaxon_pjrt.so"
    if not os.path.exists(so_path):
        return
    lib = ctypes.CDLL(so_path)
    if not hasattr(lib, "axon_start_nrt_profile"):
        return
    lib.axon_start_nrt_profile.argtypes = [
        ctypes.POINTER(ctypes.c_int64), ctypes.c_size_t]
    lib.axon_start_nrt_profile.restype = ctypes.c_int64
    lib.axon_stop_nrt_profile.argtypes = [ctypes.c_char_p]
    lib.axon_stop_nrt_profile.restype = ctypes.c_int64

    @contextlib.contextmanager
    def _hook(output_dir, device_ids):
        import jax
        jax.devices()
        if device_ids:
            ids = (ctypes.c_int64 * len(device_ids))(*device_ids)
            rc = lib.axon_start_nrt_profile(ids, len(device_ids))
        else:
            rc = lib.axon_start_nrt_profile(None, 0)
        if rc != 0:
            raise RuntimeError(f"axon_start_nrt_profile rc={rc}")
        try:
            yield
        finally:
            n = lib.axon_stop_nrt_profile(str(output_dir).encode())
            print(f"ntff profile: {n} file(s) written to {output_dir}")

    mod.set_axon_ntff_profile_hook(_hook)


def _install_wait_split():
    import concourse.bass_utils as bass_utils
    import concourse.bass2jax as bass2jax

    if getattr(bass_utils, "_wait_split_installed", False):
        return
    orig = bass_utils.compile_bir_kernel
    ctr = [0]

    def _split(bir_json: bytes) -> bytes:
        d = json.loads(bir_json)
        changed = False
        for fn in d.get("functions", []):
            for bb_ in fn.get("blocks", []):
                new = []
                for inst in bb_.get("instructions", []):
                    si = inst.get("sync_info") or {}
                    ow = si.get("on_wait") or []
                    if len(ow) > 1:
                        changed = True
                        for w in ow[:-1]:
                            ctr[0] += 1
                            new.append({
                                "debug": inst.get("debug", 0),
                                "engine": inst["engine"],
                                "ins": [],
                                "name": f"WSPLIT-{ctr[0]}",
                                "opcode": "NoOp",
                                "outs": [],
                                "sync_info": {"on_update": [], "on_wait": [w]},
                            })
                        si["on_wait"] = [ow[-1]]
                    new.append(inst)
                bb_["instructions"] = new
        return json.dumps(d).encode() if changed else bir_json

    def patched(bir_json, tmpdir, neff_name="file.neff"):
        return orig(_split(bir_json), tmpdir, neff_name=neff_name)

    bass_utils.compile_bir_kernel = patched
    bass2jax.compile_bir_kernel = patched

    # let walrus drop redundant LDWEIGHTS for repeated stationary operands
    if os.environ.get("LDW_OPT", "0") == "1":
        orig_run = bass_utils.run_command

        def run2(cmd, **kw):
            cmd = ["--enable-ldw-opt=true" if c == "--enable-ldw-opt=false"
                   else c for c in cmd]
            return orig_run(cmd, **kw)

        bass_utils.run_command = run2
    bass_utils._wait_split_installed = True


# ---------------------------------------------------------------------------
# Builder
# ---------------------------------------------------------------------------
def _build():
    import contextlib

    import concourse.bass as bass
    import concourse.tile as tile
    from concourse import mybir

    dt = mybir.dt
    AF = mybir.ActivationFunctionType

    nc = bass.Bass("TRN2", target_bir_lowering=False, debug=False)

    xT = nc.declare_dram_parameter("xT", [D, N], dt.bfloat16, isOutput=False)
    wq = nc.declare_dram_parameter("wq", [D + 2, D], dt.bfloat16, isOutput=False)
    wk = nc.declare_dram_parameter("wk", [D + 2, D], dt.bfloat16, isOutput=False)
    wv = nc.declare_dram_parameter("wv", [D + 2, D], dt.bfloat16, isOutput=False)
    wo = nc.declare_dram_parameter("wo", [D + 1, D], dt.bfloat16, isOutput=False)
    expb = nc.declare_dram_parameter("expb", [H, N, N], dt.bfloat16, isOutput=False)
    outT = nc.declare_dram_parameter("outT", [D, N], dt.float32, isOutput=True)

    with tile.TileContext(nc) as tc:
        ctx = contextlib.ExitStack()
        with ctx:
            # ---- SBUF pools ----
            sing = ctx.enter_context(tc.tile_pool(name="sing", bufs=1))
            wp = ctx.enter_context(tc.tile_pool(name="wp", bufs=1))
            actp = ctx.enter_context(tc.tile_pool(name="actp", bufs=1))
            ebp = ctx.enter_context(tc.tile_pool(name="ebp", bufs=2))
            # xt tiles freed after the xs pass
            xt_ctx = contextlib.ExitStack()
            xtp = xt_ctx.enter_context(tc.tile_pool(name="xtp", bufs=1))
            sqp = xt_ctx.enter_context(tc.tile_pool(name="sqp", bufs=1))
            stp = xt_ctx.enter_context(tc.tile_pool(name="stp", bufs=1))
            # ---- PSUM pools ----
            # big: qk/v accumulators ([128,1024] = 2 banks), whole kernel
            pbig = ctx.enter_context(tc.tile_pool(name="pbig", bufs=1,
                                                  space="PSUM"))
            # stats psums (6 banks), freed before the head loop opens psc/pa
            st_ctx = contextlib.ExitStack()
            pst = st_ctx.enter_context(tc.tile_pool(name="pst", bufs=1,
                                                    space="PSUM"))

            # --- constants ---
            ones_col_b = sing.tile([128, 1], dt.bfloat16, tag="ones_col_b")
            nc.gpsimd.memset(ones_col_b[:], 1.0)
            ones_row = sing.tile([1, 128], dt.float32, tag="ones_row")
            nc.gpsimd.memset(ones_row[:], 1.0)
            ones128b = sing.tile([128, 64], dt.bfloat16, tag="ones128b")
            nc.gpsimd.memset(ones128b[:], 1.0)
            eps_t = sing.tile([1, 1], dt.float32, tag="eps")
            nc.gpsimd.memset(eps_t[:], 1e-5)
            dstage = sing.tile([128, 512], dt.float32, tag="dstage")
            nc.gpsimd.memset(dstage[:], 1.0)
            rcp_s = sing.tile([128, 512], dt.bfloat16, tag="rcp_s")

            # --- eb prefetch (biggest DMA stream; lo half on the gpsimd
            # queue right away, hi half on sync after the weight loads) ---
            eb_t = {}

            def issue_eb_lo(h):
                t = ebp.tile([128, NT, N], dt.bfloat16, tag="eb")
                src = expb[h].rearrange("(kt p) q -> p kt q", p=128)
                nc.gpsimd.dma_start(out=t[:, 0:4, :], in_=src[:, 0:4, :])
                eb_t[h] = t

            def issue_eb_hi(h):
                src = expb[h].rearrange("(kt p) q -> p kt q", p=128)
                nc.sync.dma_start(out=eb_t[h][:, 4:8, :], in_=src[:, 4:8, :])

            def issue_eb(h):
                issue_eb_lo(h)
                issue_eb_hi(h)

            issue_eb_lo(0)
            issue_eb_lo(1)

            # --- x tiles (bf16, single load) + weights ---
            xt = []
            for i in range(KT):
                t = xtp.tile([128, N], dt.bfloat16, tag=f"xt{i}")
                nc.sync.dma_start(out=t[:], in_=xT[i * 128:(i + 1) * 128, :])
                xt.append(t)

            def load_w(name, par):
                ts_ = []
                for t in range(KT):
                    w = wp.tile([128, D], dt.bfloat16, tag=f"{name}{t}")
                    nc.sync.dma_start(out=w[:], in_=par[t * 128:(t + 1) * 128, :])
                    ts_.append(w)
                ex_rows = par.shape[0] - D
                ex = wp.tile([ex_rows, D], dt.bfloat16, tag=f"{name}x")
                nc.sync.dma_start(out=ex[:], in_=par[D:, :])
                ts_.append(ex)
                return ts_

            wqt = load_w("wq", wq)
            wkt = load_w("wk", wk)
            wvt = load_w("wv", wv)
            wot = load_w("wo", wo)
            issue_eb_hi(0)
            issue_eb_hi(1)

            # --- pass 1: LN stats from bf16 x via ones-matmuls ---
            psum = pst.tile([1, N], dt.float32, tag="psum")
            psq = pst.tile([1, N], dt.float32, tag="psq")
            for i in range(KT):
                sq = sqp.tile([128, N], dt.bfloat16, tag="sq")
                nc.vector.tensor_mul(sq[:], xt[i][:], xt[i][:])
                for c in range(NCH):
                    cs = slice(c * 512, (c + 1) * 512)
                    nc.tensor.matmul(psum[:, cs], ones_col_b[:], xt[i][:, cs],
                                     start=(i == 0), stop=(i == KT - 1))
                    nc.tensor.matmul(psq[:, cs], ones_col_b[:], sq[:, cs],
                                     start=(i == 0), stop=(i == KT - 1))

            # 3 reusable [1,N] f32 scratch rows (each costs 4KB/partition)
            sa = stp.tile([1, N], dt.float32, tag="sa")   # mu
            sb = stp.tile([1, N], dt.float32, tag="sb")   # msq->var->rstd
            sc = stp.tile([1, N], dt.float32, tag="sc2")  # mu^2->lnv->mrs
            nc.vector.tensor_scalar_mul(sa[:], psum[:], 1.0 / D)
            nc.vector.tensor_scalar_mul(sb[:], psq[:], 1.0 / D)
            nc.vector.tensor_mul(sc[:], sa[:], sa[:])
            nc.vector.tensor_sub(sb[:], sb[:], sc[:])      # var
            # rstd = exp(-0.5*ln(var+eps)): two fast ScalarE table ops
            # instead of the slow DVE reciprocal on the critical path
            nc.scalar.activation(sc[:], sb[:], AF.Ln, bias=eps_t[:])
            nc.scalar.activation(sb[:], sc[:], AF.Exp, scale=-0.5)  # rstd
            rstd = sb
            nc.vector.tensor_mul(sc[:], sa[:], rstd[:])    # mu*rstd
            # row 0 = ones (base_partition 0 so it can be a lone matmul rhs),
            # row 1 = -mu*rstd. Engines can't write partition 1 directly
            # (32-aligned base required), so stage row 1 and DMA it in.
            extras = sing.tile([2, N], dt.bfloat16, tag="extras")
            nc.gpsimd.memset(extras[0:1, :], 1.0)
            mrs_bf = stp.tile([1, N], dt.bfloat16, tag="mrs_bf")
            nc.vector.tensor_scalar_mul(mrs_bf[:], sc[:], -1.0)
            nc.sync.dma_start(out=extras[1:2, :], in_=mrs_bf[:])

            # broadcast rstd to all 128 partitions (K=1 fp32 matmuls)
            prb = pst.tile([128, N], dt.float32, tag="prb")
            for c in range(NCH):
                cs = slice(c * 512, (c + 1) * 512)
                nc.tensor.matmul(prb[:, cs], ones_row[:], rstd[:, cs],
                                 start=True, stop=True)
            rstd_b = sing.tile([128, N], dt.bfloat16, tag="rstd_b")
            nc.scalar.copy(rstd_b[:], prb[:])

            # --- pass 2: xs = x * rstd (bf16, 2x DVE rate) ---
            xs = []
            for i in range(KT):
                x_ = actp.tile([128, N], dt.bfloat16, tag=f"xs{i}")
                nc.vector.tensor_mul(x_[:], xt[i][:], rstd_b[:])
                xs.append(x_)

            xt_ctx.close()
            st_ctx.close()
            # pools created after the xt/stats space is freed (baseline
            # pattern: allocator reuses closed-pool space for later pools)
            qkp = ctx.enter_context(tc.tile_pool(name="qkp", bufs=6))
            atp = ctx.enter_context(tc.tile_pool(name="atp", bufs=2))
            outp = ctx.enter_context(tc.tile_pool(name="outp", bufs=1))
            # head-phase PSUM pools: scores 4 banks + pa 2 banks (+big 2 = 8)
            hd_ctx = contextlib.ExitStack()
            psc = hd_ctx.enter_context(tc.tile_pool(name="psc", bufs=1,
                                                    space="PSUM"))
            pa = hd_ctx.enter_context(tc.tile_pool(name="pa", bufs=2,
                                                   space="PSUM"))

            def rhs_k(kt, cs):
                return xs[kt][:, cs] if kt < KT else extras[:, cs]

            # --- qT/kT pack projection: one "unit" = one 128-row pack ---
            qT = [None] * KT
            kT = [None] * KT

            def qk_unit(wts, dest, name, p):
                t = qkp.tile([128, N], dt.bfloat16, tag="qkT",
                             name=f"{name}{p}")
                pc = slice(p * 128, (p + 1) * 128)
                pq = pbig.tile([128, N], dt.float32, tag="big",
                               name=f"pq_{name}{p}")
                for kt_ in range(KT + 1):
                    for c in range(NCH):
                        cs = slice(c * 512, (c + 1) * 512)
                        nc.tensor.matmul(pq[:, cs], wts[kt_][:, pc],
                                         rhs_k(kt_, cs),
                                         start=(kt_ == 0), stop=(kt_ == KT))
                nc.vector.tensor_copy(t[:], pq[:])
                dest[p] = t

            # --- v unit: activations stationary, [seq, head, 64+ones] ---
            v_ext = [None] * NT

            def v_unit(s):
                vt = actp.tile([128, H, 65], dt.bfloat16, tag=f"v{s}")
                nc.gpsimd.memset(vt[:, :, 64:65], 1.0)
                ss = slice(s * 128, (s + 1) * 128)
                pv = pbig.tile([128, N], dt.float32, tag="big",
                               name=f"pv{s}")
                for kt_ in range(KT + 1):
                    lhs = xs[kt_][:, ss] if kt_ < KT else extras[:, ss]
                    for c0, cw in [(0, 512), (512, 256)]:
                        nc.tensor.matmul(pv[:, c0:c0 + cw], lhs,
                                         wvt[kt_][:, c0:c0 + cw],
                                         start=(kt_ == 0), stop=(kt_ == KT))
                nc.vector.tensor_copy(
                    vt[:, :, 0:64],
                    pv[:, 0:D].rearrange("p (h c) -> p h c", c=64))
                v_ext[s] = vt

            # prologue packs: pairs 0 and 1 need their q/k before scoring
            qk_unit(wqt, qT, "qT", 0)
            qk_unit(wkt, kT, "kT", 0)
            qk_unit(wqt, qT, "qT", 1)
            qk_unit(wkt, kT, "kT", 1)

            # filler queue, interleaved between score groups so the PE
            # stream stays dense. Ordering constraints: all v units before
            # AV_0 (pair 1's end); pack p's units before pair p's scores.
            fillers = []
            for s in range(6):
                fillers.append(("v", s))
            fillers.append(("qk", 2, 0))
            fillers.append(("qk", 2, 1))
            fillers.append(("v", 6))
            fillers.append(("v", 7))
            for p in range(3, KT):
                fillers.append(("qk", p, 0))
                fillers.append(("qk", p, 1))

            def emit_filler(item):
                if item[0] == "v":
                    v_unit(item[1])
                elif item[2] == 0:
                    qk_unit(wqt, qT, "qT", item[1])
                else:
                    qk_unit(wkt, kT, "kT", item[1])

            def drain_filler(n=1):
                for _ in range(n):
                    if fillers:
                        emit_filler(fillers.pop(0))

            def ensure_v():
                rest = [f for f in fillers if f[0] == "v"]
                for f in rest:
                    fillers.remove(f)
                    emit_filler(f)

            def ensure_qk(p):
                rest = [f for f in fillers if f[0] == "qk" and f[1] == p]
                for f in rest:
                    fillers.remove(f)
                    emit_filler(f)

            # avT accumulators (2 heads per tile, unnormalized until norm_q)
            avT = [actp.tile([128, N], dt.bfloat16, tag=f"avT{p}",
                             name=f"avT{p}") for p in range(NP)]
            at_t = {}

            def emit_scores(p, at):
                """Row-tiled scores + exp for pair p, with fillers and the
                deferred AV/norm work for earlier pairs woven in."""
                ensure_qk(p)
                for kt_ in range(NT):
                    pt = psc.tile([128, 2048], dt.float32, tag="sc")
                    ks = slice(kt_ * 128, (kt_ + 1) * 128)
                    for h2 in range(2):
                        rs2 = slice(64 * h2, 64 * h2 + 64)
                        for c in range(NCH):
                            nc.tensor.matmul(
                                pt[:, h2 * 1024 + c * 512:
                                   h2 * 1024 + (c + 1) * 512],
                                kT[p][rs2, ks],
                                qT[p][rs2, c * 512:(c + 1) * 512],
                                start=True, stop=True)
                    nc.scalar.activation(at[:, kt_, :], pt[:], AF.Exp)
                    if kt_ == 1 and p >= 2:
                        emit_norm(p - 2)
                    if kt_ in ((1, 3, 5, 7) if p < 2 else (1, 3, 5)):
                        drain_filler()
                    if kt_ == 3:
                        emit_ebmul(p, at, 0)
                emit_ebmul(p, at, 1)

            def emit_ebmul(p, at, g):
                gs = slice(4 * g, 4 * g + 4)
                for h2 in range(2):
                    h = 2 * p + h2
                    nc.vector.tensor_mul(
                        at[:, gs, h2 * 1024:(h2 + 1) * 1024],
                        at[:, gs, h2 * 1024:(h2 + 1) * 1024],
                        eb_t[h][:, gs, :])

            def emit_av(p):
                """AV for pair p (M=65: denominator rides col 64), den
                staging to 32-aligned partitions + one batched reciprocal."""
                ensure_v()
                at = at_t[p]
                for h2 in range(2):
                    h = 2 * p + h2
                    rs2 = slice(64 * h2, 64 * h2 + 64)
                    for c in range(NCH):
                        cs = slice(c * 512, (c + 1) * 512)
                        pav = pa.tile([65, 512], dt.float32, tag="pa")
                        for kt_ in range(NT):
                            nc.tensor.matmul(
                                pav[:], v_ext[kt_][:, h, :],
                                at[:, kt_, h2 * 1024 + c * 512:
                                   h2 * 1024 + (c + 1) * 512],
                                start=(kt_ == 0), stop=(kt_ == NT - 1))
                        nc.vector.tensor_copy(avT[p][rs2, cs], pav[0:64, :])
                        j = 32 * (2 * h2 + c)
                        nc.vector.tensor_copy(dstage[j:j + 1, :],
                                              pav[64:65, :])
                with nc.allow_low_precision(
                        reason="softmax denominators in bf16 are fine"):
                    nc.vector.reciprocal(rcp_s[:], dstage[:])

            def emit_norm(p):
                """Normalize avT[p] rows using the pair's staged reciprocals
                via K=1 broadcast matmuls at explicit row-groups."""
                for h2 in range(2):
                    rs2 = slice(64 * h2, 64 * h2 + 64)
                    for c in range(NCH):
                        cs = slice(c * 512, (c + 1) * 512)
                        j = 32 * (2 * h2 + c)
                        pbc = pa.tile([64, 512], dt.float32, tag="pa")
                        nc.tensor.matmul(pbc[:], ones128b[j:j + 1, :],
                                         rcp_s[j:j + 1, :],
                                         start=True, stop=True,
                                         tile_position=(j, 0))
                        nc.vector.tensor_mul(avT[p][rs2, cs],
                                             avT[p][rs2, cs], pbc[:])

            # --- head-pair loop ---
            for p in range(NP):
                if p + 1 < NP:
                    issue_eb(2 * (p + 1))
                    issue_eb(2 * (p + 1) + 1)
                at = atp.tile([128, NT, 2048], dt.bfloat16, tag="at")
                at_t[p] = at
                emit_scores(p, at)
                if p >= 1:
                    emit_av(p - 1)
                    del at_t[p - 1]
            # order matters: norm(p) must read rcp_s before av(p+1)'s
            # reciprocal overwrites it
            emit_norm(NP - 2)
            emit_av(NP - 1)
            emit_norm(NP - 1)
            drain_filler(len(fillers))

            hd_ctx.close()
            pout = ctx.enter_context(tc.tile_pool(name="pout", bufs=2,
                                                  space="PSUM"))

            # --- output projection (bias folded; transposed out) ---
            for mt in range(KT):
                mc = slice(mt * 128, (mt + 1) * 128)
                py = pout.tile([128, N], dt.float32, tag="py")
                for kt_ in range(KT + 1):
                    for c in range(NCH):
                        cs = slice(c * 512, (c + 1) * 512)
                        rhs = avT[kt_][:, cs] if kt_ < KT else extras[0:1, cs]
                        nc.tensor.matmul(py[:, cs], wot[kt_][:, mc], rhs,
                                         start=(kt_ == 0), stop=(kt_ == KT))
                ot = outp.tile([128, N], dt.float32, tag="ot")
                nc.scalar.copy(ot[:], py[:])
                nc.sync.dma_start(out=outT[mc, :], in_=ot[:])

    return nc


# ---------------------------------------------------------------------------
# Host side
# ---------------------------------------------------------------------------
def _host_prep(x, rpb, W_qkv, W_out, b_out, ln_g, ln_b):
    g = np.asarray(ln_g, F32)
    bb_ = np.asarray(ln_b, F32)
    W_qkv = np.asarray(W_qkv, F32)
    W_out = np.asarray(W_out, F32)
    b_out = np.asarray(b_out, F32)

    def make_w(W, scale=1.0):
        Wp = (g[:, None] * W) * scale
        cw = Wp.sum(axis=0, keepdims=True)              # pairs with mrs row
        cb = (bb_[:, None] * W).sum(axis=0, keepdims=True) * scale  # with ones
        return np.ascontiguousarray(np.vstack([Wp, cb, cw]).astype(bf16))

    wq = make_w(W_qkv[:, :D], 1.0 / np.sqrt(DH))
    wk = make_w(W_qkv[:, D:2 * D])
    wv = make_w(W_qkv[:, 2 * D:])
    wo = np.ascontiguousarray(np.vstack([W_out, b_out[None, :]]).astype(bf16))
    expb = np.ascontiguousarray(
        np.exp(np.asarray(rpb, F32)[0].transpose(0, 2, 1)).astype(bf16))

    shared = {"wq": wq, "wk": wk, "wv": wv, "wo": wo, "expb": expb}
    in_maps = []
    for b_i in range(B):
        m = dict(shared)
        m["xT"] = np.ascontiguousarray(np.asarray(x[b_i], F32).T.astype(bf16))
        in_maps.append(m)
    return in_maps


def kernel(x, relative_position_bias, W_qkv, W_out, b_out, ln_g, ln_b):
    _install_wait_split()
    _install_ntff_hook()
    from concourse.bass_utils import run_bass_kernel_spmd

    if "nc" not in _cache:
        _cache["nc"] = _build()
    nc = _cache["nc"]

    in_maps = _host_prep(x, relative_position_bias, W_qkv, W_out, b_out,
                         ln_g, ln_b)
    res = run_bass_kernel_spmd(nc, in_maps, core_ids=list(range(B)))
    _cache["last_result"] = res

    out = np.empty((B, N, D), F32)
    for b_i in range(B):
        out[b_i] = res.results[b_i]["outT"].T
    return out
